# revision 9
# baseline (speedup 1.0000x reference)
"""Trainium2 Bass kernel for a GAT block.

Math (after algebraic simplification of the reference):
  h[b,f,n,k] = x[b,:,f,n] @ W[:,k] + bW[k]
  s2[b,f,n]  = h[b,f,n,:] @ a2 = v.x + c0   (s1/ab cancel inside softmax)
  d[b,f,n]   = softmax_n(s2)[n] * mask[n,n]
  out[b,k,f,n] = d[b,f,n] * h[b,f,n,k] = sum_c W[c,k] (x*d)[c,f,n] + bW[k] d[f,n]

Sharding: data-parallel over batch, 4 batches per core on 8 cores.

x is shipped as fp16; all PE matmuls are fp16 (f32 PSUM accumulation); the
softmax itself runs in f32.  Output DRAM is fp16; the host upcasts to f32.

Device pipeline per (batch, 512-frame q-unit), shapes [partitions, free]:
  1. single load: x4 [128, 400] fp16: rows 32c+fsub = x[c] (96 rows),
     rows 96:128 = 1.0 (gpsimd memset); 32 fsub rows x (16 frames, 25 nodes)
  2. s2p [32, 400] (PSUM) = V128.T @ x4: per-frame scores v.x + c0
  3. softmax in fsub layout: e32 = exp(s2p) (ACT, PSUM->SBUF);
     z [32,16] = row-sums over 25-node segments (DVE); r = 1/z;
     em = e32 * md400; dd [32,400] fp16 = em * r_bc
  4. pdd [128, 400] (PSUM) = rep4.T @ dd  (PE replicates dd into 4 blocks)
  5. x4s [128, 400] fp16 = x4 * pdd  (x*d rows 0:96, d rows 96:128)
  6. 16 fp16 matmuls into 2-bank psum tiles [128, 2, 512]:
     psum = wsel[tp].T @ x4s; wsel[tp] [128,128] selects fsubs {tp, 16+tp}
     and applies [W; bW] -> out rows (2k + jj), cols (f', n)
  7. evict PAIRS of banks per op (3 DVE / 5 ACT), cast to fp16 ->
     osb [128, 16, 400]; 2 half stores per q-unit (6.4KB descriptors)

The softmax front (steps 1-3) for unit u+1 is emitted BEFORE the body
(steps 4-7) of unit u so PE never waits on the DVE softmax chain.
"""

import sys

if "/opt/trn_rl_repo" not in sys.path:
    sys.path.insert(0, "/opt/trn_rl_repo")

import numpy as np

B, C, F, N, H = 32, 3, 2048, 25, 64
NCORES = 8
BPC = B // NCORES   # batches per core
QF = 512            # frames per q-unit
NQ = F // QF        # q-units per batch
FSUB = 16           # frames per fsub row
NS = QF // FSUB     # 32 fsub rows per q-unit
FN = F * N
TW = FSUB * N       # 400, columns per tile
NT = NS // 2        # 16 tiles (of 32 frames) per q-unit

_NC_CACHE = {}


def _build_nc():
    import concourse.bass as bass
    import concourse.bacc as bacc
    import concourse.tile as tile
    from concourse import mybir

    f32 = mybir.dt.float32
    f16 = mybir.dt.float16
    MULT = mybir.AluOpType.mult
    AX = mybir.AxisListType.X
    EXP = mybir.ActivationFunctionType.Exp

    nc = bacc.Bacc()
    x_d = nc.declare_dram_parameter("x", [BPC, C, F, N], f16, isOutput=False)
    wsel_d = nc.declare_dram_parameter("wsel", [128, NT, 128], f16, isOutput=False)
    rep4_d = nc.declare_dram_parameter("rep4", [NS, 128], f16, isOutput=False)
    v128_d = nc.declare_dram_parameter("v128", [128, NS], f16, isOutput=False)
    md_d = nc.declare_dram_parameter("md400", [NS, TW], f32, isOutput=False)
    out_d = nc.declare_dram_parameter("out", [BPC, H, F, N], f16, isOutput=True)

    with tile.TileContext(nc) as tc:
        with (
            tc.tile_pool(name="singles", bufs=1) as singles,
            tc.tile_pool(name="sm", bufs=3) as sm_pool,
            tc.tile_pool(name="x4", bufs=5) as x4_pool,
            tc.tile_pool(name="osb", bufs=3) as osb_pool,
            tc.tile_pool(name="ps", bufs=3, space="PSUM") as ps_pool,
            tc.tile_pool(name="psb", bufs=1, space="PSUM") as psb_pool,
        ):
            # front-critical singles first (v128/md400 gate the first front);
            # wsel/rep4 go on the scalar ring so they don't delay the first load
            v128_sb = singles.tile([128, NS], f16)
            nc.sync.dma_start(out=v128_sb[:], in_=v128_d[:, :])
            md_sb = singles.tile([NS, TW], f32)
            nc.sync.dma_start(out=md_sb[:], in_=md_d[:, :])
            wsel_sb = singles.tile([128, NT, 128], f16)
            nc.scalar.dma_start(out=wsel_sb[:], in_=wsel_d[:, :, :])
            rep4_sb = singles.tile([NS, 128], f16)
            nc.scalar.dma_start(out=rep4_sb[:], in_=rep4_d[:, :])

            units = [(b, q) for b in range(BPC) for q in range(NQ)]
            NU = len(units)

            def emit_load(u):
                """Issue the x4 input DMA for unit u; return the tile."""
                b, q = u
                f0 = q * QF
                base = x_d[b, :, f0 : f0 + 1, :]  # for offset only
                x4 = x4_pool.tile([128, TW], f16, tag="x4")
                nc.gpsimd.memset(x4[96:128, :], 1.0)
                src4 = bass.AP(
                    tensor=base.tensor,
                    offset=base.offset,
                    ap=[[FN, C], [TW, NS], [1, TW]],
                )
                nc.sync.dma_start(out=x4[0:96, :], in_=src4)
                return x4

            def emit_front(x4):
                """Scores + softmax for one unit; return dd [32, 400] fp16."""
                s2p = psb_pool.tile([32, 512], f32, tag="s2p")
                nc.tensor.matmul(
                    s2p[:, 0:TW], v128_sb[:], x4[:], start=True, stop=True
                )
                e32 = sm_pool.tile([NS, TW], f32, tag="e32")
                nc.scalar.activation(out=e32[:], in_=s2p[:, 0:TW], func=EXP)
                ev = e32[:].rearrange("p (a b) -> p a b", b=N)
                z = sm_pool.tile([NS, FSUB], f32, tag="z")
                nc.vector.reduce_sum(out=z[:], in_=ev, axis=AX)
                r = sm_pool.tile([NS, FSUB], f32, tag="r")
                nc.vector.reciprocal(out=r[:], in_=z[:])
                em = sm_pool.tile([NS, TW], f32, tag="em")
                nc.gpsimd.tensor_tensor(out=em[:], in0=e32[:], in1=md_sb[:], op=MULT)
                dd = sm_pool.tile([NS, TW], f16, tag="dd")
                rr = r[:, :]
                r_bc = bass.AP(
                    tensor=rr.tensor,
                    offset=rr.offset,
                    ap=[rr.ap[0], [1, FSUB], [0, N]],
                )
                nc.vector.tensor_tensor(out=dd[:], in0=em[:], in1=r_bc, op=MULT)
                return dd

            # eviction engine per pair, alternating 3/4 DVE by unit parity
            EV3 = ["v", "a", "a", "v", "a", "a", "v", "a"]
            EV4 = ["v", "a", "v", "a", "v", "a", "v", "a"]

            PF = 3  # load prefetch depth
            loads = [emit_load(units[i]) for i in range(min(PF, NU))]
            fronts = [emit_front(loads[0])]
            for ui, u in enumerate(units):
                b, q = u
                f0 = q * QF
                x4 = loads.pop(0)
                dd = fronts.pop(0)
                if ui + PF < NU:
                    loads.append(emit_load(units[ui + PF]))
                if ui + 1 < NU:
                    fronts.append(emit_front(loads[0]))
                # ---- 4. pdd [128, 400] = rep4.T @ dd  (fp16 PE)
                pdd = psb_pool.tile([128, 512], f32, tag="pdd")
                nc.tensor.matmul(
                    pdd[:, 0:TW], rep4_sb[:], dd[:], start=True, stop=True
                )
                # ---- 5. x4s = x4 * pdd  (cast to fp16 on write)
                x4s = x4_pool.tile([128, TW], f16, tag="x4s")
                nc.vector.tensor_tensor(
                    out=x4s[:], in0=x4[:], in1=pdd[:, 0:TW], op=MULT
                )
                # ---- 6./7. 16 fp16 matmuls + paired evictions + stores
                osb = osb_pool.tile([128, NT, TW], f16)
                for tp in range(NT):
                    j = tp % 2
                    if j == 0:
                        ph2 = ps_pool.tile([128, 2, 512], f32, tag="ph2")
                    nc.tensor.matmul(
                        ph2[:, j, 0:TW],
                        wsel_sb[:, tp, :],
                        x4s[:, :],
                        start=True,
                        stop=True,
                    )
                    if j == 1:
                        pair = tp // 2
                        if EV3[pair] == "v":
                            nc.vector.tensor_copy(
                                osb[:, tp - 1 : tp + 1, :], ph2[:, :, 0:TW]
                            )
                        else:
                            nc.scalar.copy(
                                osb[:, tp - 1 : tp + 1, :], ph2[:, :, 0:TW]
                            )
                    if tp % 8 == 7:
                        hh = tp // 8
                        osl = out_d[b, :, f0 : f0 + 1, :]
                        dst = bass.AP(
                            tensor=osl.tensor,
                            offset=osl.offset + hh * 8 * TW,
                            ap=[[FN, H], [16 * TW, 2], [1, 8 * TW]],
                        )
                        nc.sync.dma_start(
                            out=dst,
                            in_=osb[:, 8 * hh : 8 * (hh + 1), :],
                        )
    nc.compile()
    return nc


def _get_nc():
    if "nc" not in _NC_CACHE:
        _NC_CACHE["nc"] = _build_nc()
    return _NC_CACHE["nc"]


def _make_in_maps(x, mask, W, bW, a1, a2, ab):
    x = np.asarray(x, np.float32)
    mask = np.asarray(mask, np.float32)
    W = np.asarray(W, np.float32)
    bW = np.asarray(bW, np.float32)
    a2 = np.asarray(a2, np.float32)

    v = (W @ a2).astype(np.float32)                    # [C]
    c0 = np.float32(bW @ a2)
    md = np.diag(mask).astype(np.float32)              # [N]

    # wsel[row = 32 c + fsub, tp, col = 2 k + jj]:
    #   delta[fsub == tp + 16 jj] * (W[c, k] if c < 3 else bW[k])
    # (column order (k, jj)-interleaved so the store DMA is affine)
    wsel = np.zeros((128, NT, 128), np.float32)
    cols = np.arange(H)
    for tp in range(NT):
        for jj in range(2):
            fsub = tp + 16 * jj
            for c in range(3):
                wsel[32 * c + fsub, tp, 2 * cols + jj] = W[c]
            wsel[96 + fsub, tp, 2 * cols + jj] = bW
    rep4 = np.zeros((NS, 128), np.float32)
    for blk in range(4):
        rep4[:, 32 * blk : 32 * (blk + 1)] = np.eye(NS, dtype=np.float32)
    # v128[row = 32 c + fsub, fsub] = v[c] (c < 3), c0 (ones rows)
    v128 = np.zeros((128, NS), np.float32)
    for fsub in range(NS):
        for c in range(3):
            v128[32 * c + fsub, fsub] = v[c]
        v128[96 + fsub, fsub] = c0
    md400 = np.tile(md[None, :], (NS, FSUB)).astype(np.float32)  # [32, 400]

    x16 = np.ascontiguousarray(x.astype(np.float16))
    in_maps = []
    for cix in range(NCORES):
        in_maps.append(
            {
                "x": np.ascontiguousarray(x16[cix * BPC : (cix + 1) * BPC]),
                "wsel": wsel.astype(np.float16),
                "rep4": rep4.astype(np.float16),
                "v128": v128.astype(np.float16),
                "md400": md400,
            }
        )
    return in_maps


def run(x, mask, W, bW, a1, a2, ab, **run_kwargs):
    from concourse.bass_utils import run_bass_kernel_spmd

    nc = _get_nc()
    in_maps = _make_in_maps(x, mask, W, bW, a1, a2, ab)
    res = run_bass_kernel_spmd(nc, in_maps, core_ids=list(range(NCORES)), **run_kwargs)
    out = np.concatenate(
        [np.asarray(res.results[i]["out"]) for i in range(NCORES)], axis=0
    ).astype(np.float32)
    return out, res


def kernel(x, mask, W, bW, a1, a2, ab):
    out, _ = run(x, mask, W, bW, a1, a2, ab)
    return out


# revision 17
# speedup vs baseline: 1.2440x; 1.2440x over previous
"""Trainium2 Bass kernel for a GAT block.

Math (after algebraic simplification of the reference):
  h[b,f,n,k] = x[b,:,f,n] @ W[:,k] + bW[k]
  s2[b,f,n]  = h[b,f,n,:] @ a2 = v.x + c0   (s1/ab cancel inside softmax)
  d[b,f,n]   = softmax_n(s2)[n] * mask[n,n]
  out[b,k,f,n] = d[b,f,n] * h[b,f,n,k] = sum_c W[c,k] (x*d)[c,f,n] + bW[k] d[f,n]

Sharding: data-parallel over batch, 4 batches per core on 8 cores.

x is shipped as fp16; all PE matmuls are fp16 (f32 PSUM accumulation); the
softmax itself runs in f32.  Output DRAM is fp16; the host upcasts to f32.

Device pipeline per (batch, 512-frame q-unit), shapes [partitions, free]:
  1. single load: x4 [128, 400] fp16: rows 32c+fsub = x[c] (96 rows),
     rows 96:128 = 1.0 (gpsimd memset); 32 fsub rows x (16 frames, 25 nodes)
  2. scores in a 32-row slice of the persistent s2p PSUM bank:
     s2p[32u':32u'+32] = V128.T @ x4  (u' = unit mod 4)
  3. softmax in fsub layout: e32 = exp(s2p slice) (ACT, PSUM->SBUF);
     z [32,16] = row-sums over 25-node segments (DVE); r = 1/z;
     em = e32 * md400; dd [32,400] fp16 = em * r_bc
  4. pdd [128, 400] (PSUM) = rep4.T @ dd  (PE replicates dd into 4 blocks)
  5. x4s [128, 400] fp16 = x4 * pdd  (x*d rows 0:96, d rows 96:128)
  6. 16 fp16 matmuls into 2-bank psum tiles [128, 2, 512]:
     psum = wsel[tp].T @ x4s; wsel[tp] [128,128] selects fsubs {tp, 16+tp}
     and applies [W; bW] -> out rows (2k + jj), cols (f', n)
  7. evict PAIRS of banks per op (3 DVE / 5 ACT), cast to fp16 ->
     osb [128, 16, 400]; 2 half stores per q-unit (6.4KB descriptors)

The softmax front (steps 1-3) runs TWO units ahead of the body (steps 4-7)
so PE never waits on the DVE softmax chain and the startup ramp is short.
"""

import sys

if "/opt/trn_rl_repo" not in sys.path:
    sys.path.insert(0, "/opt/trn_rl_repo")

import numpy as np

B, C, F, N, H = 32, 3, 2048, 25, 64
NCORES = 8
BPC = B // NCORES   # batches per core
QF = 512            # frames per q-unit
NQ = F // QF        # q-units per batch
FSUB = 16           # frames per fsub row
NS = QF // FSUB     # 32 fsub rows per q-unit
FN = F * N
TW = FSUB * N       # 400, columns per tile
NT = NS // 2        # 16 tiles (of 32 frames) per q-unit

_NC_CACHE = {}


def _build_nc():
    import concourse.bass as bass
    import concourse.bacc as bacc
    import concourse.tile as tile
    from concourse import mybir

    f32 = mybir.dt.float32
    f16 = mybir.dt.float16
    MULT = mybir.AluOpType.mult
    AX = mybir.AxisListType.X
    EXP = mybir.ActivationFunctionType.Exp

    nc = bacc.Bacc()
    x_d = nc.declare_dram_parameter("x", [BPC, C, F, N], f16, isOutput=False)
    wsel_d = nc.declare_dram_parameter("wsel", [128, NT, 128], f16, isOutput=False)
    rep4_d = nc.declare_dram_parameter("rep4", [NS, 128], f16, isOutput=False)
    v128_d = nc.declare_dram_parameter("v128", [128, NS], f16, isOutput=False)
    md_d = nc.declare_dram_parameter("md400", [NS, TW], f32, isOutput=False)
    out_d = nc.declare_dram_parameter("out", [BPC, H, F, N], f16, isOutput=True)

    with tile.TileContext(nc) as tc:
        with (
            tc.tile_pool(name="singles", bufs=1) as singles,
            tc.tile_pool(name="sm", bufs=3) as sm_pool,
            tc.tile_pool(name="x4", bufs=5) as x4_pool,
            tc.tile_pool(name="osb", bufs=3) as osb_pool,
            tc.tile_pool(name="ps", bufs=3, space="PSUM") as ps_pool,
            tc.tile_pool(name="psb", bufs=1, space="PSUM") as psb_pool,
        ):
            # front-critical singles first (v128/md400 gate the first front);
            # wsel/rep4 go on the scalar ring so they don't delay the first load
            v128_sb = singles.tile([128, NS], f16)
            nc.sync.dma_start(out=v128_sb[:], in_=v128_d[:, :])
            md_sb = singles.tile([NS, TW], f32)
            nc.sync.dma_start(out=md_sb[:], in_=md_d[:, :])
            wsel_sb = singles.tile([128, NT, 128], f16)
            nc.scalar.dma_start(out=wsel_sb[:], in_=wsel_d[:, :, :])
            rep4_sb = singles.tile([NS, 128], f16)
            nc.scalar.dma_start(out=rep4_sb[:], in_=rep4_d[:, :])

            # persistent scores bank: 3 units cycle through 32-row slices
            s2p = psb_pool.tile([128, 512], f32, tag="s2p")

            units = [(b, q) for b in range(BPC) for q in range(NQ)]
            NU = len(units)

            def emit_load(u):
                """Issue the x4 input DMA for unit u; return the tile."""
                b, q = u
                f0 = q * QF
                base = x_d[b, :, f0 : f0 + 1, :]  # for offset only
                x4 = x4_pool.tile([128, TW], f16, tag="x4")
                nc.gpsimd.memset(x4[96:128, :], 1.0)
                src4 = bass.AP(
                    tensor=base.tensor,
                    offset=base.offset,
                    ap=[[FN, C], [TW, NS], [1, TW]],
                )
                nc.sync.dma_start(out=x4[0:96, :], in_=src4)
                return x4

            def emit_front(x4, ui):
                """Scores + softmax for unit index ui; return dd [32,400] f16."""
                p0 = 32 * (ui % 3)
                sl = s2p[p0 : p0 + 32, 0:TW]
                nc.tensor.matmul(sl, v128_sb[:], x4[:], start=True, stop=True)
                e32 = sm_pool.tile([NS, TW], f32, tag="e32")
                nc.scalar.activation(out=e32[:], in_=sl, func=EXP)
                ev = e32[:].rearrange("p (a b) -> p a b", b=N)
                z = sm_pool.tile([NS, FSUB], f32, tag="z")
                nc.vector.reduce_sum(out=z[:], in_=ev, axis=AX)
                r = sm_pool.tile([NS, FSUB], f32, tag="r")
                nc.vector.reciprocal(out=r[:], in_=z[:])
                em = sm_pool.tile([NS, TW], f32, tag="em")
                nc.vector.tensor_tensor(out=em[:], in0=e32[:], in1=md_sb[:], op=MULT)
                dd = sm_pool.tile([NS, TW], f16, tag="dd")
                rr = r[:, :]
                r_bc = bass.AP(
                    tensor=rr.tensor,
                    offset=rr.offset,
                    ap=[rr.ap[0], [1, FSUB], [0, N]],
                )
                nc.vector.tensor_tensor(out=dd[:], in0=em[:], in1=r_bc, op=MULT)
                return dd

            # eviction engine per pair: 3 DVE / 5 ACT
            EV = ["v", "a", "v", "a", "a", "v", "a", "a"]

            PF = 3  # load prefetch depth
            FD = 2  # front pipelining depth
            loads = [emit_load(units[i]) for i in range(min(PF, NU))]
            fronts = [emit_front(loads[i], i) for i in range(min(FD, NU))]
            for ui, u in enumerate(units):
                b, q = u
                f0 = q * QF
                x4 = loads.pop(0)
                dd = fronts.pop(0)
                if ui + PF < NU:
                    loads.append(emit_load(units[ui + PF]))
                if ui + FD < NU:
                    fronts.append(emit_front(loads[FD - 1], ui + FD))
                # ---- 4. pdd [128, 400] = rep4.T @ dd  (PE broadcast, fp16)
                pdd = psb_pool.tile([128, 512], f32, tag="pdd")
                nc.tensor.matmul(
                    pdd[:, 0:TW], rep4_sb[:], dd[:], start=True, stop=True
                )
                # ---- 5. x4s = x4 * pdd  (cast to fp16 on write)
                x4s = x4_pool.tile([128, TW], f16, tag="x4s")
                nc.vector.tensor_tensor(
                    out=x4s[:], in0=x4[:], in1=pdd[:, 0:TW], op=MULT
                )
                # ---- 6./7. 16 fp16 matmuls + paired evictions + stores
                osb = osb_pool.tile([128, NT, TW], f16)
                for tp in range(NT):
                    j = tp % 2
                    if j == 0:
                        ph2 = ps_pool.tile([128, 2, 512], f32, tag="ph2")
                    nc.tensor.matmul(
                        ph2[:, j, 0:TW],
                        wsel_sb[:, tp, :],
                        x4s[:, :],
                        start=True,
                        stop=True,
                    )
                    if j == 1:
                        pair = tp // 2
                        if EV[pair] == "v":
                            nc.vector.tensor_copy(
                                osb[:, tp - 1 : tp + 1, :], ph2[:, :, 0:TW]
                            )
                        else:
                            nc.scalar.copy(
                                osb[:, tp - 1 : tp + 1, :], ph2[:, :, 0:TW]
                            )
                    if tp % 8 == 7:
                        hh = tp // 8
                        osl = out_d[b, :, f0 : f0 + 1, :]
                        dst = bass.AP(
                            tensor=osl.tensor,
                            offset=osl.offset + hh * 8 * TW,
                            ap=[[FN, H], [16 * TW, 2], [1, 8 * TW]],
                        )
                        nc.sync.dma_start(
                            out=dst,
                            in_=osb[:, 8 * hh : 8 * (hh + 1), :],
                        )
    nc.compile()
    return nc


def _get_nc():
    if "nc" not in _NC_CACHE:
        _NC_CACHE["nc"] = _build_nc()
    return _NC_CACHE["nc"]


def _make_in_maps(x, mask, W, bW, a1, a2, ab):
    x = np.asarray(x, np.float32)
    mask = np.asarray(mask, np.float32)
    W = np.asarray(W, np.float32)
    bW = np.asarray(bW, np.float32)
    a2 = np.asarray(a2, np.float32)

    v = (W @ a2).astype(np.float32)                    # [C]
    c0 = np.float32(bW @ a2)
    md = np.diag(mask).astype(np.float32)              # [N]

    # wsel[row = 32 c + fsub, tp, col = 2 k + jj]:
    #   delta[fsub == tp + 16 jj] * (W[c, k] if c < 3 else bW[k])
    # (column order (k, jj)-interleaved so the store DMA is affine)
    wsel = np.zeros((128, NT, 128), np.float32)
    cols = np.arange(H)
    for tp in range(NT):
        for jj in range(2):
            fsub = tp + 16 * jj
            for c in range(3):
                wsel[32 * c + fsub, tp, 2 * cols + jj] = W[c]
            wsel[96 + fsub, tp, 2 * cols + jj] = bW
    rep4 = np.zeros((NS, 128), np.float32)
    for blk in range(4):
        rep4[:, 32 * blk : 32 * (blk + 1)] = np.eye(NS, dtype=np.float32)
    # v128[row = 32 c + fsub, fsub] = v[c] (c < 3), c0 (ones rows)
    v128 = np.zeros((128, NS), np.float32)
    for fsub in range(NS):
        for c in range(3):
            v128[32 * c + fsub, fsub] = v[c]
        v128[96 + fsub, fsub] = c0
    md400 = np.tile(md[None, :], (NS, FSUB)).astype(np.float32)  # [32, 400]

    x16 = np.ascontiguousarray(x.astype(np.float16))
    in_maps = []
    for cix in range(NCORES):
        in_maps.append(
            {
                "x": np.ascontiguousarray(x16[cix * BPC : (cix + 1) * BPC]),
                "wsel": wsel.astype(np.float16),
                "rep4": rep4.astype(np.float16),
                "v128": v128.astype(np.float16),
                "md400": md400,
            }
        )
    return in_maps


def run(x, mask, W, bW, a1, a2, ab, **run_kwargs):
    from concourse.bass_utils import run_bass_kernel_spmd

    nc = _get_nc()
    in_maps = _make_in_maps(x, mask, W, bW, a1, a2, ab)
    res = run_bass_kernel_spmd(nc, in_maps, core_ids=list(range(NCORES)), **run_kwargs)
    out = np.concatenate(
        [np.asarray(res.results[i]["out"]) for i in range(NCORES)], axis=0
    ).astype(np.float32)
    return out, res


def kernel(x, mask, W, bW, a1, a2, ab):
    out, _ = run(x, mask, W, bW, a1, a2, ab)
    return out


# revision 21
# speedup vs baseline: 1.2867x; 1.0344x over previous
"""Trainium2 Bass kernel for a GAT block.

Math (after algebraic simplification of the reference):
  h[b,f,n,k] = x[b,:,f,n] @ W[:,k] + bW[k]
  s2[b,f,n]  = h[b,f,n,:] @ a2 = v.x + c0   (s1/ab cancel inside softmax)
  d[b,f,n]   = softmax_n(s2)[n] * mask[n,n]
  out[b,k,f,n] = d[b,f,n] * h[b,f,n,k] = sum_c W[c,k] (x*d)[c,f,n] + bW[k] d[f,n]

Sharding: data-parallel over batch, 4 batches per core on 8 cores.

x is shipped as fp16; all PE matmuls are fp16 (f32 PSUM accumulation); the
softmax itself runs in f32.  Output DRAM is fp16; the host upcasts to f32.

Device pipeline per (batch, 512-frame q-unit), shapes [partitions, free]:
  1. single load: x4 [128, 400] fp16: rows 32c+fsub = x[c] (96 rows),
     rows 96:128 = 1.0 (gpsimd memset); 32 fsub rows x (16 frames, 25 nodes)
  2. scores in a 32-row slice of the persistent s2p PSUM bank:
     s2p[32u':32u'+32] = V128.T @ x4  (u' = unit mod 4)
  3. softmax in fsub layout: e32 = exp(s2p slice) (ACT, PSUM->SBUF);
     z [32,16] = row-sums over 25-node segments (DVE); r = 1/z;
     em = e32 * md400; dd [32,400] fp16 = em * r_bc
  4. pdd [128, 400] (PSUM) = rep4.T @ dd  (PE replicates dd into 4 blocks)
  5. x4s [128, 400] fp16 = x4 * pdd  (x*d rows 0:96, d rows 96:128)
  6. 16 fp16 matmuls into 2-bank psum tiles [128, 2, 512]:
     psum = wsel[tp].T @ x4s; wsel[tp] [128,128] selects fsubs {tp, 16+tp}
     and applies [W; bW] -> out rows (2k + jj), cols (f', n)
  7. evict PAIRS of banks per op (3 DVE / 5 ACT), cast to fp16 ->
     osb [128, 16, 400]; 2 half stores per q-unit (6.4KB descriptors)

The softmax front (steps 1-3) runs TWO units ahead of the body (steps 4-7)
so PE never waits on the DVE softmax chain and the startup ramp is short.
"""

import sys

if "/opt/trn_rl_repo" not in sys.path:
    sys.path.insert(0, "/opt/trn_rl_repo")

import numpy as np

B, C, F, N, H = 32, 3, 2048, 25, 64
NCORES = 8
BPC = B // NCORES   # batches per core
QF = 512            # frames per q-unit
NQ = F // QF        # q-units per batch
FSUB = 16           # frames per fsub row
NS = QF // FSUB     # 32 fsub rows per q-unit
FN = F * N
TW = FSUB * N       # 400, columns per tile
NT = NS // 2        # 16 tiles (of 32 frames) per q-unit

_NC_CACHE = {}


def _build_nc():
    import concourse.bass as bass
    import concourse.bacc as bacc
    import concourse.tile as tile
    from concourse import mybir

    f32 = mybir.dt.float32
    f16 = mybir.dt.float16
    MULT = mybir.AluOpType.mult
    AX = mybir.AxisListType.X
    EXP = mybir.ActivationFunctionType.Exp

    nc = bacc.Bacc()
    x_d = nc.declare_dram_parameter("x", [BPC, C, F, N], f16, isOutput=False)
    wsel_d = nc.declare_dram_parameter("wsel", [128, NT, 128], f16, isOutput=False)
    rep4_d = nc.declare_dram_parameter("rep4", [NS, 128], f16, isOutput=False)
    v128_d = nc.declare_dram_parameter("v128", [128, NS], f16, isOutput=False)
    md_d = nc.declare_dram_parameter("md400", [NS, TW], f32, isOutput=False)
    out_d = nc.declare_dram_parameter("out", [BPC, H, F, N], f16, isOutput=True)

    with tile.TileContext(nc) as tc:
        with (
            tc.tile_pool(name="singles", bufs=1) as singles,
            tc.tile_pool(name="sm", bufs=3) as sm_pool,
            tc.tile_pool(name="x4", bufs=5) as x4_pool,
            tc.tile_pool(name="osb", bufs=3) as osb_pool,
            tc.tile_pool(name="ps", bufs=3, space="PSUM") as ps_pool,
            tc.tile_pool(name="psb", bufs=1, space="PSUM") as psb_pool,
        ):
            # singles ride the scalar ring so the sync ring starts on x4 loads
            v128_sb = singles.tile([128, NS], f16)
            nc.scalar.dma_start(out=v128_sb[:], in_=v128_d[:, :])
            md_sb = singles.tile([NS, TW], f32)
            nc.scalar.dma_start(out=md_sb[:], in_=md_d[:, :])
            wsel_sb = singles.tile([128, NT, 128], f16)
            nc.scalar.dma_start(out=wsel_sb[:], in_=wsel_d[:, :, :])
            rep4_sb = singles.tile([NS, 128], f16)
            nc.scalar.dma_start(out=rep4_sb[:], in_=rep4_d[:, :])

            # persistent scores bank: 3 units cycle through 32-row slices
            s2p = psb_pool.tile([128, 512], f32, tag="s2p")

            units = [(b, q) for b in range(BPC) for q in range(NQ)]
            NU = len(units)

            def emit_load(u):
                """Issue the x4 input DMA for unit u; return the tile."""
                b, q = u
                f0 = q * QF
                base = x_d[b, :, f0 : f0 + 1, :]  # for offset only
                x4 = x4_pool.tile([128, TW], f16, tag="x4")
                nc.gpsimd.memset(x4[96:128, :], 1.0)
                src4 = bass.AP(
                    tensor=base.tensor,
                    offset=base.offset,
                    ap=[[FN, C], [TW, NS], [1, TW]],
                )
                nc.sync.dma_start(out=x4[0:96, :], in_=src4)
                return x4

            def emit_front(x4, ui):
                """Scores + softmax for unit index ui; return dd [32,400] f16."""
                p0 = 32 * (ui % 3)
                sl = s2p[p0 : p0 + 32, 0:TW]
                nc.tensor.matmul(sl, v128_sb[:], x4[:], start=True, stop=True)
                e32 = sm_pool.tile([NS, TW], f32, tag="e32")
                nc.scalar.activation(out=e32[:], in_=sl, func=EXP)
                ev = e32[:].rearrange("p (a b) -> p a b", b=N)
                z = sm_pool.tile([NS, FSUB], f32, tag="z")
                nc.vector.reduce_sum(out=z[:], in_=ev, axis=AX)
                r = sm_pool.tile([NS, FSUB], f32, tag="r")
                nc.vector.reciprocal(out=r[:], in_=z[:])
                em = sm_pool.tile([NS, TW], f32, tag="em")
                nc.gpsimd.tensor_tensor(out=em[:], in0=e32[:], in1=md_sb[:], op=MULT)
                dd = sm_pool.tile([NS, TW], f16, tag="dd")
                rr = r[:, :]
                r_bc = bass.AP(
                    tensor=rr.tensor,
                    offset=rr.offset,
                    ap=[rr.ap[0], [1, FSUB], [0, N]],
                )
                nc.vector.tensor_tensor(out=dd[:], in0=em[:], in1=r_bc, op=MULT)
                return dd

            # eviction engine per pair: 3 DVE / 5 ACT, 4/4 on odd units
            EV3 = ["v", "a", "v", "a", "a", "v", "a", "a"]
            EV4 = ["v", "a", "v", "a", "v", "a", "a", "a"]

            PF = 3  # load prefetch depth
            FD = 2  # front pipelining depth
            loads = [emit_load(units[i]) for i in range(min(PF, NU))]
            fronts = [emit_front(loads[i], i) for i in range(min(FD, NU))]
            for ui, u in enumerate(units):
                b, q = u
                f0 = q * QF
                x4 = loads.pop(0)
                dd = fronts.pop(0)
                if ui + PF < NU:
                    loads.append(emit_load(units[ui + PF]))
                if ui + FD < NU:
                    fronts.append(emit_front(loads[FD - 1], ui + FD))
                # ---- 4. pdd [128, 400] = rep4.T @ dd  (PE broadcast, fp16)
                pdd = psb_pool.tile([128, 512], f32, tag="pdd")
                nc.tensor.matmul(
                    pdd[:, 0:TW], rep4_sb[:], dd[:], start=True, stop=True
                )
                # ---- 5. x4s = x4 * pdd  (cast to fp16 on write)
                x4s = x4_pool.tile([128, TW], f16, tag="x4s")
                nc.vector.tensor_tensor(
                    out=x4s[:], in0=x4[:], in1=pdd[:, 0:TW], op=MULT
                )
                # ---- 6./7. 16 fp16 matmuls + paired evictions + stores
                osb = osb_pool.tile([128, NT, TW], f16)
                for tp in range(NT):
                    j = tp % 2
                    if j == 0:
                        ph2 = ps_pool.tile([128, 2, 512], f32, tag="ph2")
                    nc.tensor.matmul(
                        ph2[:, j, 0:TW],
                        wsel_sb[:, tp, :],
                        x4s[:, :],
                        start=True,
                        stop=True,
                    )
                    if j == 1:
                        pair = tp // 2
                        EV = EV4 if ui % 2 else EV3
                        if EV[pair] == "v":
                            nc.vector.tensor_copy(
                                osb[:, tp - 1 : tp + 1, :], ph2[:, :, 0:TW]
                            )
                        else:
                            nc.scalar.copy(
                                osb[:, tp - 1 : tp + 1, :], ph2[:, :, 0:TW]
                            )
                    # store in halves; quarters on the last unit to cut drain
                    SW = 4 if ui == NU - 1 else 8
                    if tp % SW == SW - 1:
                        hh = tp // SW
                        osl = out_d[b, :, f0 : f0 + 1, :]
                        dst = bass.AP(
                            tensor=osl.tensor,
                            offset=osl.offset + hh * SW * TW,
                            ap=[[FN, H], [16 * TW, 2], [1, SW * TW]],
                        )
                        nc.sync.dma_start(
                            out=dst,
                            in_=osb[:, SW * hh : SW * (hh + 1), :],
                        )
    nc.compile()
    return nc


def _get_nc():
    if "nc" not in _NC_CACHE:
        _NC_CACHE["nc"] = _build_nc()
    return _NC_CACHE["nc"]


def _make_in_maps(x, mask, W, bW, a1, a2, ab):
    x = np.asarray(x, np.float32)
    mask = np.asarray(mask, np.float32)
    W = np.asarray(W, np.float32)
    bW = np.asarray(bW, np.float32)
    a2 = np.asarray(a2, np.float32)

    v = (W @ a2).astype(np.float32)                    # [C]
    c0 = np.float32(bW @ a2)
    md = np.diag(mask).astype(np.float32)              # [N]

    # wsel[row = 32 c + fsub, tp, col = 2 k + jj]:
    #   delta[fsub == tp + 16 jj] * (W[c, k] if c < 3 else bW[k])
    # (column order (k, jj)-interleaved so the store DMA is affine)
    wsel = np.zeros((128, NT, 128), np.float32)
    cols = np.arange(H)
    for tp in range(NT):
        for jj in range(2):
            fsub = tp + 16 * jj
            for c in range(3):
                wsel[32 * c + fsub, tp, 2 * cols + jj] = W[c]
            wsel[96 + fsub, tp, 2 * cols + jj] = bW
    rep4 = np.zeros((NS, 128), np.float32)
    for blk in range(4):
        rep4[:, 32 * blk : 32 * (blk + 1)] = np.eye(NS, dtype=np.float32)
    # v128[row = 32 c + fsub, fsub] = v[c] (c < 3), c0 (ones rows)
    v128 = np.zeros((128, NS), np.float32)
    for fsub in range(NS):
        for c in range(3):
            v128[32 * c + fsub, fsub] = v[c]
        v128[96 + fsub, fsub] = c0
    md400 = np.tile(md[None, :], (NS, FSUB)).astype(np.float32)  # [32, 400]

    x16 = np.ascontiguousarray(x.astype(np.float16))
    in_maps = []
    for cix in range(NCORES):
        in_maps.append(
            {
                "x": np.ascontiguousarray(x16[cix * BPC : (cix + 1) * BPC]),
                "wsel": wsel.astype(np.float16),
                "rep4": rep4.astype(np.float16),
                "v128": v128.astype(np.float16),
                "md400": md400,
            }
        )
    return in_maps


def run(x, mask, W, bW, a1, a2, ab, **run_kwargs):
    from concourse.bass_utils import run_bass_kernel_spmd

    nc = _get_nc()
    in_maps = _make_in_maps(x, mask, W, bW, a1, a2, ab)
    res = run_bass_kernel_spmd(nc, in_maps, core_ids=list(range(NCORES)), **run_kwargs)
    out = np.concatenate(
        [np.asarray(res.results[i]["out"]) for i in range(NCORES)], axis=0
    ).astype(np.float32)
    return out, res


def kernel(x, mask, W, bW, a1, a2, ab):
    out, _ = run(x, mask, W, bW, a1, a2, ab)
    return out
